# revision 10
# baseline (speedup 1.0000x reference)
"""MHSA Trainium2 kernel: B=4, S=2048, D=1024, H=16 heads of 64.

Sharding (8 cores): core c -> batch b=c//2, head-group g=c%2 (8 heads = 512
proj cols). Identical program on every core; only the data differs.

Per-core layouts (host pre-transposes, so no on-device transposes at all):
  xT  [1024, 2048] = x[b].T          wqT/wkT/wvT [1024, 512] = W[cols].T
  woT [512, 1024]  = Wo[:, cols].T   out [2048, 1024] partial (host sums pairs)

Device program:
  qT = wqT.T @ xT + bq   (1/8 score scale pre-folded into wqT/bq on host)
  kT = wkT.T @ xT + bk
  v  = xT.T @ wvT        (v bias deferred: bv @ woT added on host)
  per head h: sT = kT[h].T @ qT[h]; P = exp(sT)  [keys on partitions]
    PV with ones-augmented V: out[65, s] = [V_h | 1].T @ P  -> row 64 = sums
    attnT[h] = PV[0:64] * (1/sums broadcast)
  out_partial = attnT.T @ woT    (DMA'd straight from PSUM)
"""

import os
from contextlib import ExitStack

import numpy as np

import concourse.bass as bass
import concourse.mybir as mybir


def _install_ntff_shim():
    """The agent image's `antenv` lacks `axon_hooks`, which
    run_bass_kernel_spmd imports when trace=True under axon. Provide it,
    wired to the ctypes NTFF hook from trn_agent_boot when available."""
    import sys
    import types
    try:
        from antenv import axon_hooks  # noqa: F401
        return
    except ImportError:
        pass
    try:
        mod = types.ModuleType("antenv.axon_hooks")
        mod._hook = None
        mod.set_axon_ntff_profile_hook = lambda h: setattr(mod, "_hook", h)
        mod.get_axon_ntff_profile_hook = lambda: mod._hook
        import antenv
        sys.modules["antenv.axon_hooks"] = mod
        antenv.axon_hooks = mod
        try:
            from trn_agent_boot.trn_boot import _ntff_profile_via_ctypes
            import os.path
            so = "/opt/axon/libaxon_pjrt.so"
            if os.path.exists(so):
                mod._hook = _ntff_profile_via_ctypes(so)
        except Exception:
            pass
    except Exception:
        pass


_install_ntff_shim()
import concourse.tile as tile
from concourse import bacc
from concourse.bass_utils import run_bass_kernel_spmd

F32 = mybir.dt.float32
F32R = mybir.dt.float32r

S = 2048       # sequence (rows per core's batch)
DF = 1024      # full model dim (contraction for projections)
J = 512        # proj cols per core (8 heads x 64)
HEADS = 8
HD = 64
N_CORES = 8

LAST_RESULT = {}


def _mm(nc, out, lhsT, rhs, mm_dt, **kw):
    nc.tensor.matmul(out, lhsT, rhs, **kw)


def _build(mm_dt):
    MDT = F32 if mm_dt is None else mm_dt
    nc = bacc.Bacc(None, target_bir_lowering=False, debug=False)

    xT_d = nc.declare_dram_parameter("xT", [DF, S], MDT, False)
    wqT_d = nc.declare_dram_parameter("wqT", [DF, J], MDT, False)
    wkT_d = nc.declare_dram_parameter("wkT", [DF, J], MDT, False)
    wvT_d = nc.declare_dram_parameter("wvT", [DF, J], MDT, False)
    bq_d = nc.declare_dram_parameter("bq", [J], F32, False)
    bk_d = nc.declare_dram_parameter("bk", [J], F32, False)
    woT_d = nc.declare_dram_parameter("woT", [J, DF], MDT, False)
    ones_d = nc.declare_dram_parameter("ones", [128, HEADS], MDT, False)
    out_d = nc.declare_dram_parameter("out", [S, DF], F32, isOutput=True)

    with tile.TileContext(nc) as tc, ExitStack() as ctx:
        persist = ctx.enter_context(tc.tile_pool(name="persist", bufs=1))
        # persistent SBUF tensors
        qT = [persist.tile([128, S], MDT, name=f"qT{i}", tag=f"qT{i}") for i in range(4)]
        kT = [persist.tile([128, S], MDT, name=f"kT{i}", tag=f"kT{i}") for i in range(4)]
        # v tiles laid out [128 keys, head, 65] so lhsT [:, h, :] is the
        # ones-augmented per-head V block
        vt = [persist.tile([128, HEADS, HD + 1], MDT, name=f"v{i}", tag=f"v{i}") for i in range(16)]
        bq_sb = persist.tile([128, 4], F32, tag="bq")
        bk_sb = persist.tile([128, 4], F32, tag="bk")

        nc.sync.dma_start(out=bq_sb, in_=bq_d[:].rearrange("(a p) -> p a", p=128))
        nc.sync.dma_start(out=bk_sb, in_=bk_d[:].rearrange("(a p) -> p a", p=128))

        # ---------------- Phase A: projections (one pass over x) ----------
        with tc.tile_pool(name="wqk", bufs=1) as wqk, \
             tc.tile_pool(name="xs", bufs=9) as xs, \
             tc.tile_pool(name="psA", bufs=3, space="PSUM") as psA:
            wq_sb = [wqk.tile([128, J], MDT, name=f"wq{k}", tag=f"wq{k}") for k in range(8)]
            wk_sb = [wqk.tile([128, J], MDT, name=f"wk{k}", tag=f"wk{k}") for k in range(8)]
            wv_sb = [wqk.tile([128, J], MDT, name=f"wv{k}", tag=f"wv{k}") for k in range(8)]
            for k in range(8):
                nc.sync.dma_start(out=wq_sb[k], in_=wqT_d[128 * k:128 * (k + 1), :])
                nc.sync.dma_start(out=wk_sb[k], in_=wkT_d[128 * k:128 * (k + 1), :])
                nc.sync.dma_start(out=wv_sb[k], in_=wvT_d[128 * k:128 * (k + 1), :])

            for sc in range(4):       # 512-wide s-chunks
                ss = slice(512 * sc, 512 * (sc + 1))
                xt = [xs.tile([128, 512], MDT, name="xt", tag="xt") for _ in range(8)]
                for kc in range(8):
                    nc.sync.dma_start(out=xt[kc], in_=xT_d[128 * kc:128 * (kc + 1), ss])
                for jt in range(4):   # qT / kT output partition tiles
                    jj = slice(128 * jt, 128 * (jt + 1))
                    q_ps = psA.tile([128, 512], F32, name="psA", tag="psA")
                    for kc in range(8):
                        _mm(nc, q_ps, wq_sb[kc][:, jj], xt[kc], mm_dt,
                            start=(kc == 0), stop=(kc == 7))
                    nc.vector.tensor_scalar_add(qT[jt][:, ss], q_ps, bq_sb[:, jt:jt + 1])
                    k_ps = psA.tile([128, 512], F32, name="psA", tag="psA")
                    for kc in range(8):
                        _mm(nc, k_ps, wk_sb[kc][:, jj], xt[kc], mm_dt,
                            start=(kc == 0), stop=(kc == 7))
                    nc.vector.tensor_scalar_add(kT[jt][:, ss], k_ps, bk_sb[:, jt:jt + 1])
                for stl in range(4):  # v output tiles for this s-chunk
                    st = 4 * sc + stl
                    v_ps = psA.tile([128, 512], F32, name="psA", tag="psA")
                    for kc in range(8):
                        _mm(nc, v_ps, xt[kc][:, 128 * stl:128 * (stl + 1)], wv_sb[kc],
                            mm_dt, start=(kc == 0), stop=(kc == 7))
                    nc.vector.tensor_copy(
                        vt[st][:, :, 0:HD],
                        v_ps[:].rearrange("p (h d) -> p h d", h=HEADS))
                    nc.sync.dma_start(
                        out=vt[st][:, :, HD:HD + 1],
                        in_=ones_d[:].rearrange("p (a b) -> p a b", b=1))

        # ---------------- Phase B + C: attention then out-proj ------------
        with tc.tile_pool(name="wo", bufs=1) as wo, \
             tc.tile_pool(name="pt", bufs=3) as ptp, \
             tc.tile_pool(name="rbc", bufs=2) as rbcp, \
             tc.tile_pool(name="attn", bufs=1) as attnp, \
             tc.tile_pool(name="tmp", bufs=2) as tmpp, \
             tc.tile_pool(name="dscr", bufs=4, space="DRAM") as dscr, \
             tc.tile_pool(name="psS", bufs=2, space="PSUM") as psS, \
             tc.tile_pool(name="psPV", bufs=1, space="PSUM") as psPV, \
             tc.tile_pool(name="psO", bufs=2, space="PSUM") as psO:
            wo_sb = [wo.tile([128, DF], MDT, name=f"wo{i}", tag=f"wo{i}") for i in range(4)]
            for i in range(4):
                nc.sync.dma_start(out=wo_sb[i], in_=woT_d[128 * i:128 * (i + 1), :])
            attnT = [attnp.tile([128, S], MDT, name=f"at{i}", tag=f"at{i}") for i in range(4)]

            for sc2 in range(2):      # 1024-wide s-chunks
                s0 = 1024 * sc2
                for h in range(HEADS):
                    jt, ro = h // 2, 64 * (h % 2)
                    rows = slice(ro, ro + 64)
                    pv_ps = psPV.tile([65, 1024], F32, name="pv", tag="pv")
                    for kt in range(16):
                        tt = slice(128 * kt, 128 * (kt + 1))
                        s_ps = psS.tile([128, 1024], F32, name="sps", tag="sps")
                        _mm(nc, s_ps[:, 0:512], kT[jt][rows, tt],
                            qT[jt][rows, s0:s0 + 512], mm_dt)
                        _mm(nc, s_ps[:, 512:1024], kT[jt][rows, tt],
                            qT[jt][rows, s0 + 512:s0 + 1024], mm_dt)
                        pt = ptp.tile([128, 1024], MDT, name="pt", tag="pt")
                        nc.scalar.activation(pt, s_ps, mybir.ActivationFunctionType.Exp)
                        _mm(nc, pv_ps[:, 0:512], vt[kt][:, h, :], pt[:, 0:512],
                            mm_dt, start=(kt == 0), stop=(kt == 15))
                        _mm(nc, pv_ps[:, 512:1024], vt[kt][:, h, :], pt[:, 512:1024],
                            mm_dt, start=(kt == 0), stop=(kt == 15))
                    # 1/sums: reciprocal of psum row 64 into SBUF, then
                    # DMA-replicate across 128 partitions
                    rrow = rbcp.tile([1, 1024], F32, name="rrow", tag="rrow")
                    nc.vector.reciprocal(rrow, pv_ps[64:65, :])
                    rd = dscr.tile([1, 1024], F32, name="rd", tag="rd")
                    nc.sync.dma_start(out=rd, in_=rrow)
                    rec = rbcp.tile([128, 1024], F32, name="rec", tag="rec")
                    nc.sync.dma_start(out=rec, in_=rd.partition_broadcast(128))
                    if ro == 0:
                        nc.vector.tensor_mul(
                            attnT[jt][0:64, s0:s0 + 1024], pv_ps[0:64, :], rec[0:64, :])
                    else:
                        hop = tmpp.tile([64, 1024], MDT, name="hop", tag="hop")
                        nc.vector.tensor_mul(hop, pv_ps[0:64, :], rec[0:64, :])
                        nc.sync.dma_start(out=attnT[jt][64:128, s0:s0 + 1024], in_=hop)

                for st8 in range(8):  # out-proj for this 1024-wide s region
                    st = 8 * sc2 + st8
                    sl = slice(128 * st, 128 * (st + 1))
                    for oc in range(2):
                        ocs = slice(512 * oc, 512 * (oc + 1))
                        o_ps = psO.tile([128, 512], F32, name="ops", tag="ops")
                        for jc in range(4):
                            _mm(nc, o_ps, attnT[jc][:, sl], wo_sb[jc][:, ocs],
                                mm_dt, start=(jc == 0), stop=(jc == 3))
                        o_sb = tmpp.tile([128, 512], F32, name="osb", tag="osb")
                        nc.vector.tensor_copy(o_sb, o_ps)
                        nc.sync.dma_start(out=out_d[sl, ocs], in_=o_sb)
    nc.compile()
    return nc


_NC_CACHE = {}


def _get_nc(mm_dt):
    key = str(mm_dt)
    if key not in _NC_CACHE:
        _NC_CACHE[key] = _build(mm_dt)
    return _NC_CACHE[key]


def kernel(**inputs):
    x = np.asarray(inputs["x"], np.float32)
    Wq = np.asarray(inputs["Wq"], np.float32)
    bq = np.asarray(inputs["bq"], np.float32)
    Wk = np.asarray(inputs["Wk"], np.float32)
    bk = np.asarray(inputs["bk"], np.float32)
    Wv = np.asarray(inputs["Wv"], np.float32)
    bv = np.asarray(inputs["bv"], np.float32)
    Wo = np.asarray(inputs["Wo"], np.float32)
    bo = np.asarray(inputs["bo"], np.float32)

    scale = np.float32(1.0 / np.sqrt(HD))
    mm_dt = {"f32": None, "f32r": F32R}[os.environ.get("BASS_MM_DT", "f32r")]
    nc = _get_nc(mm_dt)

    in_maps = []
    bvwo = []     # host-side bv @ woT rows, one per core
    for c in range(N_CORES):
        b, g = c // 2, c % 2
        cols = slice(J * g, J * (g + 1))
        woTs = np.ascontiguousarray(Wo[:, cols].T)
        in_maps.append({
            "xT": np.ascontiguousarray(x[b].T),
            "wqT": np.ascontiguousarray(Wq[cols, :].T) * scale,
            "wkT": np.ascontiguousarray(Wk[cols, :].T),
            "wvT": np.ascontiguousarray(Wv[cols, :].T),
            "bq": np.ascontiguousarray(bq[cols]) * scale,
            "bk": np.ascontiguousarray(bk[cols]),
            "woT": woTs,
            "ones": np.ones((128, HEADS), np.float32),
            "out": np.zeros((S, DF), np.float32),
        })
        bvwo.append(bv[cols] @ woTs)
    for m in in_maps:
        m.pop("out")

    res = run_bass_kernel_spmd(
        nc, in_maps, list(range(N_CORES)),
        trace=bool(os.environ.get("BASS_TRACE")))
    LAST_RESULT["exec_time_ns"] = res.exec_time_ns
    LAST_RESULT["mean_exec_time_ns"] = getattr(res, "mean_exec_time_ns", None)
    LAST_RESULT["profile_json"] = res.profile_json
    it = res.instructions_and_trace
    LAST_RESULT["trace_path"] = it[1] if it else None
    LAST_RESULT["insts"] = it[0] if it else None

    B = x.shape[0]
    out = np.empty((B, S, DF), np.float32)
    for b in range(B):
        out[b] = (res.results[2 * b]["out"] + res.results[2 * b + 1]["out"]
                  + bvwo[2 * b][None, :] + bvwo[2 * b + 1][None, :]
                  + bo[None, :])
    return out
